# revision 1
# baseline (speedup 1.0000x reference)
"""CARAFE-naive upsampling (N=4, C=256, H=W=64, k=5, g=4, s=2) on 8 TRN2
NeuronCores.

Strategy
--------
Sharding: core c <- (batch n = c//2, group-pair j = c%2). Each core owns 128
feature channels (2 of the 4 mask groups) of one batch image.

Compute: the per-pixel mask application is reformulated as TensorEngine
matmuls. For one source row r and a w-tile of 32 source columns:

    out[(g,c), (a,w,b)] += sum_{w'} statT[(g,w'), (g,c)] * B[(g,w'), (a,w,b)]

where statT is the (block-diagonal over the 2 groups) transposed feature row
and B is a *banded* matrix holding mask values on shifted diagonals
(row w+dj pairs source column w0+w+dj-2 with output column w). The 5 row
offsets di accumulate into PSUM (start/stop accumulation groups).

B cannot be built on-device (its diagonal layout is not an affine access
pattern), so the host pre-shears masks into B in numpy and ships it to HBM
in matmul-ready bf16 layout. bf16 inflation is 7.2x over raw masks but the
TensorEngine then does all 52M MACs/core in ~628 matmuls.
"""

import sys

import numpy as np

for _p in ("/opt/trn_rl_repo", "/opt/pypackages"):
    if _p not in sys.path:
        sys.path.append(_p)

import ml_dtypes  # noqa: E402
from contextlib import ExitStack  # noqa: E402

import concourse.bass as bass  # noqa: E402
import concourse.tile as tile  # noqa: E402
from concourse import bacc, mybir  # noqa: E402
from concourse.bass_utils import run_bass_kernel_spmd  # noqa: E402

# Problem constants (hardcoded per harness contract)
KS = 5            # kernel size
G = 4             # mask groups
S = 2             # upscale
N, C, H, W = 4, 256, 64, 64
Wt = 32           # w-tile
NT = W // Wt      # 2 tiles
KB = Wt + 4       # band rows per group
KK = 2 * KB       # contraction dim = 72
BF16 = ml_dtypes.bfloat16

_NC_CACHE = {}


def _build_bass():
    # Bacc (not raw Bass): its finalize() runs generate_event_semaphores,
    # which splits multi-sem waits to satisfy the 1-wait-per-instruction
    # TRN2 ISA constraint.
    nc = bacc.Bacc()
    # k-major layouts: every DMA walks contiguous bytes per SBUF partition.
    # stat ships 96 rows (72 real + 24 host-zero); rows [96:128) are
    # memset on-device — disjoint from the DMA rows, so no WAW chain.
    stat_d = nc.declare_dram_parameter(
        "stat", [96, H, NT, 128], mybir.dt.bfloat16, isOutput=False)
    # B: one tile per output row pair h — fine-grained deps pipeline best.
    # The first 4 tiles ship with padding rows (bmat0): no memset gates
    # the first matmuls.
    bmat0_d = nc.declare_dram_parameter(
        "bmat0", [4, 128, KS, NT, 128], mybir.dt.bfloat16, isOutput=False)
    bmat_d = nc.declare_dram_parameter(
        "bmat", [H - 4, KK, KS, NT, 128], mybir.dt.bfloat16, isOutput=False)
    out_d = nc.declare_dram_parameter(
        "out", [128, S * H, S * W], mybir.dt.bfloat16, isOutput=True)

    NSLOT = 22   # B tile slots
    HB = 4       # output rows per batched store
    out_rows = out_d.rearrange("c (hb y) x -> c hb (y x)", hb=H // HB)

    with tile.TileContext(nc) as tc, ExitStack() as ctx:
        statp = ctx.enter_context(tc.tile_pool(name="statp", bufs=1))
        bp = ctx.enter_context(tc.tile_pool(name="bp", bufs=NSLOT))
        pp = ctx.enter_context(tc.tile_pool(name="pp", bufs=8, space="PSUM"))
        op = ctx.enter_context(tc.tile_pool(name="op", bufs=3))

        btiles = {}
        psums = {}
        otiles = {}

        def load_b(h):
            bt = bp.tile([128, KS, NT, 128], mybir.dt.bfloat16,
                         name=f"bt{h}", tag="bt")
            if h < 4:
                # padding rows come from DRAM (host-zeroed)
                eng = nc.sync if h % 2 == 0 else nc.scalar
                eng.dma_start(out=bt, in_=bmat0_d[h])
            else:
                # padded K rows multiply zeroed stationary rows; memset
                # keeps them finite (NaN x 0 = NaN) on the first pass
                # through a slot; slots reuse rows [72:128) untouched.
                if h < NSLOT:
                    (nc.vector if h % 2 == 0 else nc.gpsimd).memset(
                        bt[64:128], 0.0)
                # HWDGE lanes only (SWDGE descriptor gen is ~1us each)
                eng = nc.sync if h % 2 == 0 else nc.scalar
                eng.dma_start(out=bt[0:KK], in_=bmat_d[h - 4])
            btiles[h] = bt

        # Stationaries in SBUF; K padded 72->128 so LDWEIGHTS gets FWL
        # (needs NumWeights==128). 4-row chunks, loaded lazily in
        # first-use order: HW DMA queues are FIFOs, so anything emitted
        # ahead of a tile delays every consumer of that tile; small
        # chunks keep the queues fine-grained.
        stats = [None] * (H // 4)

        def load_stat(rb, eng):
            st = statp.tile([128, 4, NT, 128], mybir.dt.bfloat16,
                            name=f"st{rb}", tag=f"st{rb}")
            # pad rows: finite AND zero (they face B's garbage pad rows);
            # disjoint from the DMA rows [0:96) so they run in parallel
            nc.gpsimd.memset(st[96:128], 0.0)
            eng.dma_start(out=st[0:96],
                          in_=stat_d[:, 4 * rb: 4 * rb + 4])
            stats[rb] = st

        load_stat(0, nc.scalar)  # gates MM #1 — first on its queue
        load_b(0)
        load_b(1)
        load_b(2)
        for h in range(3, NSLOT):  # warm the remaining B slots
            load_b(h)
        load_stat(1, nc.sync)

        def rfirst(h):
            return max(0, h - 2)

        def rlast(h):
            return min(H - 1, h + 2)

        for r in range(H):
            # prefetch B tiles well ahead of the live window
            for h in range(max(0, r - 2), min(H - 1, r + 17) + 1):
                if h not in btiles:
                    load_b(h)
            # stat chunk for rows [4rb, 4rb+4) emitted ~10 rows ahead
            rb_need = min(H // 4 - 1, (r + 10) // 4)
            if stats[rb_need] is None:
                load_stat(rb_need, nc.sync if rb_need % 2 else nc.scalar)

            # t-outer: 5 consecutive matmuls share one stationary
            for t in range(NT):
                for di in range(KS):
                    h = r + 2 - di
                    if not (0 <= h < H):
                        continue
                    if h not in psums:
                        psums[h] = pp.tile([128, NT, 128], mybir.dt.float32,
                                           name=f"ps{h}", tag="ps")
                    nc.tensor.matmul(
                        out=psums[h][:, t, :],
                        lhsT=stats[r // 4][:, r % 4, t, :],
                        rhs=btiles[h][:, di, t, :],
                        start=(r == rfirst(h) and t == 0),
                        stop=(r == rlast(h) and t == NT - 1),
                        skip_group_check=True,
                    )

            # drain finished output rows into a 4-row staging buffer
            done = [r - 2] if r - 2 >= 0 else []
            if r == H - 1:
                done += [H - 2, H - 1]
            for h in done:
                blk = h // HB
                if blk not in otiles:
                    otiles[blk] = op.tile([128, HB, S, NT, Wt, S],
                                          mybir.dt.bfloat16,
                                          name=f"ot{blk}", tag="ot")
                ot_tawb = otiles[blk][:, h % HB].rearrange(
                    "c a t w b -> c t a w b")
                ps_tawb = psums[h].rearrange(
                    "c t (a w b) -> c t a w b", a=S, b=S)
                # copies on DVE (otherwise idle)
                nc.vector.tensor_copy(out=ot_tawb, in_=ps_tawb)
                del psums[h], btiles[h]
                if h % HB == HB - 1:
                    # 2KB/partition contiguous store, queues alternating
                    eng = nc.sync if blk % 2 == 0 else nc.scalar
                    eng.dma_start(out=out_rows[:, blk], in_=otiles[blk])
                    del otiles[blk]

    nc.finalize()
    return nc


def _host_shards(features, masks):
    """Build per-core stat/bmat arrays (bf16)."""
    in_maps = []
    iw = np.arange(Wt)
    for c in range(8):
        n, j = c // 2, c % 2
        f = features[n, 128 * j: 128 * (j + 1)]        # [128, 64, 64] f32
        m = masks[n, 50 * j: 50 * j + 50]              # [50, 128, 128] f32

        # stationaries: stat[g*KB + w', r, t, g*64 + cc] = fpad[g*64+cc, r, 32t+w']
        # rows [KK:96) stay zero (K padding); [96:128) zeroed on-device
        stat = np.zeros((96, H, NT, 128), np.float32)
        fp = np.pad(f, ((0, 0), (0, 0), (2, 2)))
        for g in range(2):
            for t in range(NT):
                sl = fp[g * 64:(g + 1) * 64, :, Wt * t: Wt * t + KB]
                stat[g * KB:(g + 1) * KB, :, t, g * 64:(g + 1) * 64] = \
                    sl.transpose(2, 1, 0)

        # banded masks: B[h, g*KB + w + dj, di, t, (a,w,b)]
        M8 = m.reshape(2, KS, KS, H, S, NT, Wt, S)     # g,di,dj,h,a,t,w,b
        B2 = np.zeros((H, KS, NT, S, S, KK, Wt), np.float32)
        for g in range(2):
            for dj in range(KS):
                src = M8[g, :, dj].transpose(1, 0, 3, 2, 5, 4)  # h,di,t,a,b,w
                B2[:, :, :, :, :, g * KB + iw + dj, iw] = src
        # [h, KK, di, t, (a,w,b)] per-row tiles
        ball = np.ascontiguousarray(
            B2.transpose(0, 5, 1, 2, 3, 6, 4)).reshape(H, KK, KS, NT, 128)
        bmat = ball[4:]
        bmat0 = np.zeros((4, 128, KS, NT, 128), np.float32)
        bmat0[:, 0:KK] = ball[:4]

        in_maps.append({
            "stat": np.ascontiguousarray(stat).astype(BF16),
            "bmat0": bmat0.astype(BF16),
            "bmat": np.ascontiguousarray(bmat).astype(BF16),
        })
    return in_maps


def kernel(features, masks, _trace=False):
    features = np.asarray(features, dtype=np.float32)
    masks = np.asarray(masks, dtype=np.float32)

    in_maps = _host_shards(features, masks)

    if "nc" not in _NC_CACHE:
        _NC_CACHE["nc"] = _build_bass()
    nc = _NC_CACHE["nc"]

    res = run_bass_kernel_spmd(nc, in_maps, list(range(8)), trace=_trace)
    kernel._last_result = res

    out = np.empty((N, C, S * H, S * W), np.float32)
    for c in range(8):
        n, j = c // 2, c % 2
        out[n, 128 * j: 128 * (j + 1)] = \
            res.results[c]["out"].astype(np.float32)
    return out



# revision 2
# speedup vs baseline: 1.2358x; 1.2358x over previous
"""CARAFE-naive upsampling (N=4, C=256, H=W=64, k=5, g=4, s=2) on 8 TRN2
NeuronCores.

Strategy
--------
Sharding: core c <- (batch n = c//2, group-pair j = c%2). Each core owns 128
feature channels (2 of the 4 mask groups) of one batch image.

Compute: blocked im2col. The output is tiled into 4x4 source blocks; each
block's 25-tap neighborhood lives in an 8x8 source window (K=64). Per tile
(hb, wb) ONE matmul computes every tap in a single pass:

    psum[(g,h',a,w,b), c] = sum_{(r,w'')} statT[(r,w''), (g,h',a,w,b)]
                                        * feat[(r,w''), c]

where statT is the host-sheared mask tile (each column holds an output
pixel's 25 taps placed at its window offsets; 64/25 = 2.56x inflation) and
feat is the host-im2col'd feature window (shared by both groups). The two
group-diagonal halves of psum are real output; the off-group halves are
discarded by the copy-out. 256 matmuls/core total; HBM traffic 12.6 MB/core
(vs 19.7 MB for the banded-row formulation) and one LDWEIGHTS+MATMUL pair
per tile keeps the PE stream at ~81 ns/tile.
"""

import sys

import numpy as np
from numpy.lib.stride_tricks import sliding_window_view

for _p in ("/opt/trn_rl_repo", "/opt/pypackages"):
    if _p not in sys.path:
        sys.path.append(_p)

import ml_dtypes  # noqa: E402
from contextlib import ExitStack  # noqa: E402

import concourse.bass as bass  # noqa: E402
import concourse.tile as tile  # noqa: E402
from concourse import bacc, mybir  # noqa: E402
from concourse.bass_utils import run_bass_kernel_spmd  # noqa: E402

# Problem constants (hardcoded per harness contract)
N, C, H, W = 4, 256, 64, 64
NB = 16          # blocks per spatial axis (4x4 source pixels each)
K = 64           # contraction = 8x8 source window
BF16 = ml_dtypes.bfloat16

_NC_CACHE = {}


def _build_bass():
    nc = bacc.Bacc()
    # Per-hb slabs: [K=64 part, wb, 128] -> 4KB contiguous per partition,
    # 256KB per DMA.
    stat_d = nc.declare_dram_parameter(
        "stat", [NB, K, NB, 128], mybir.dt.bfloat16, isOutput=False)
    feat_d = nc.declare_dram_parameter(
        "feat", [NB, K, NB, 128], mybir.dt.bfloat16, isOutput=False)
    out_d = nc.declare_dram_parameter(
        "out", [NB, 128, NB, 64], mybir.dt.bfloat16, isOutput=True)

    with tile.TileContext(nc) as tc, ExitStack() as ctx:
        sp = ctx.enter_context(tc.tile_pool(name="sp", bufs=6))
        fp = ctx.enter_context(tc.tile_pool(name="fp", bufs=6))
        # psum quad = [128, 4, 128] f32 = 2KB/partition = one full bank
        pp = ctx.enter_context(tc.tile_pool(name="pp", bufs=6, space="PSUM"))
        op = ctx.enter_context(tc.tile_pool(name="op", bufs=3))

        for hb in range(NB):
            st = sp.tile([K, NB, 128], mybir.dt.bfloat16,
                         name=f"s{hb}", tag="st")
            ft = fp.tile([K, NB, 128], mybir.dt.bfloat16,
                         name=f"f{hb}", tag="ft")
            # loads on the sync HWDGE queue, stores on scalar's -> the two
            # rings drain independently
            nc.sync.dma_start(out=st, in_=stat_d[hb])
            nc.sync.dma_start(out=ft, in_=feat_d[hb])
            ot = op.tile([128, NB, 64], mybir.dt.bfloat16,
                         name=f"o{hb}", tag="ot")
            for q in range(NB // 4):
                ps = pp.tile([128, 4, 128], mybir.dt.float32,
                             name=f"p{hb}_{q}", tag="ps")
                for i in range(4):
                    wb = 4 * q + i
                    nc.tensor.matmul(
                        out=ps[:, i, :],
                        lhsT=st[:, wb, :],
                        rhs=ft[:, wb, :],
                        start=True, stop=True,
                        skip_group_check=True,
                    )
                # group-diagonal halves only; off-group halves are garbage
                nc.vector.tensor_copy(
                    out=ot[0:64, 4 * q: 4 * q + 4, :],
                    in_=ps[0:64, :, 0:64])
                nc.scalar.copy(
                    out=ot[64:128, 4 * q: 4 * q + 4, :],
                    in_=ps[64:128, :, 64:128])
            nc.scalar.dma_start(out=out_d[hb], in_=ot)

    nc.finalize()
    return nc


def _host_shards(features, masks):
    """Build per-core stat/feat arrays (bf16)."""
    in_maps = []
    for core in range(8):
        n, j = core // 2, core % 2
        f = features[n, 128 * j: 128 * (j + 1)]        # [128, 64, 64] f32
        m = masks[n, 50 * j: 50 * j + 50].reshape(2, 25, 128, 128)

        # feature im2col: feat[hb, (r,w''), wb, c] = Fpad[c, 4hb+r, 4wb+w'']
        fpad = np.pad(f, ((0, 0), (2, 2), (2, 2)))
        sw = sliding_window_view(fpad, (8, 8), axis=(1, 2))[:, ::4, ::4]
        feat = np.ascontiguousarray(
            sw.transpose(1, 3, 4, 2, 0)).reshape(NB, K, NB, 128)

        # mask shear: stat[hb, (r,w''), wb, (g,h',a,w,b)] holds tap
        # (di=r-h', dj=w''-w) of output pixel (2(4hb+h')+a, 2(4wb+w)+b)
        mm = m.reshape(2, 5, 5, NB, 4, 2, NB, 4, 2)  # g,di,dj,hb,h',a,wb,w,b
        stat = np.zeros((NB, 8, 8, NB, 2, 4, 2, 4, 2), np.float32)
        for di in range(5):
            for dj in range(5):
                for hp in range(4):
                    for w in range(4):
                        stat[:, hp + di, w + dj, :, :, hp, :, w, :] = \
                            mm[:, di, dj, :, hp, :, :, w, :].transpose(
                                1, 3, 0, 2, 4)
        stat = stat.reshape(NB, K, NB, 128)

        in_maps.append({
            "stat": stat.astype(BF16),
            "feat": feat.astype(BF16),
        })
    return in_maps


def kernel(features, masks, _trace=False):
    features = np.asarray(features, dtype=np.float32)
    masks = np.asarray(masks, dtype=np.float32)

    in_maps = _host_shards(features, masks)

    if "nc" not in _NC_CACHE:
        _NC_CACHE["nc"] = _build_bass()
    nc = _NC_CACHE["nc"]

    res = run_bass_kernel_spmd(nc, in_maps, list(range(8)), trace=_trace)
    kernel._last_result = res

    out = np.empty((N, C, 2 * H, 2 * W), np.float32)
    for core in range(8):
        n, j = core // 2, core % 2
        od = res.results[core]["out"].astype(np.float32)
        od = od.reshape(NB, 2, 4, 2, 4, 2, NB, 64)   # hb,g,h',a,w,b,wb,cc
        od = od.transpose(1, 7, 0, 2, 3, 6, 4, 5)    # g,cc,hb,h',a,wb,w,b
        out[n, 128 * j: 128 * (j + 1)] = od.reshape(128, 128, 128)
    return out


# revision 3
# speedup vs baseline: 1.2466x; 1.0087x over previous
"""CARAFE-naive upsampling (N=4, C=256, H=W=64, k=5, g=4, s=2) on 8 TRN2
NeuronCores.

Strategy
--------
Sharding: core c <- (batch n = c//2, group-pair j = c%2). Each core owns 128
feature channels (2 of the 4 mask groups) of one batch image.

Compute: blocked im2col. The output is tiled into 4x8 source blocks; a
block's 25-tap neighborhood lives in an 8x12 source window (K=96). Per
(tile, group) ONE matmul computes every tap in a single pass:

    psum[(h',a,w,b), c] = sum_{(r,w'')} statT[(r,w''), (h',a,w,b)]
                                      * feat[(r,w''), c]

statT is the host-sheared mask tile (each column holds one output pixel's
25 taps placed at its window offsets; 96/25 = 3.8x inflation over raw
masks); feat is the host-im2col'd feature window (shared by both groups,
N=64 channel columns each). Every psum element is real output: full
[128, 512] PSUM banks drain with one contiguous DVE/ACT copy per
(row-block, group). Everything is SBUF-resident; all 32 load DMAs are
issued up front on the two HWDGE queues so the 512 LDWEIGHTS+MATMUL
stream never waits on HBM; stores ride the otherwise-idle GpSimd SWDGE
queue. ~13.6 MB HBM traffic and 512 PE instructions per core.
"""

import sys

import numpy as np
from numpy.lib.stride_tricks import sliding_window_view

for _p in ("/opt/trn_rl_repo", "/opt/pypackages"):
    if _p not in sys.path:
        sys.path.append(_p)

import ml_dtypes  # noqa: E402
from contextlib import ExitStack  # noqa: E402

import concourse.bass as bass  # noqa: E402
import concourse.tile as tile  # noqa: E402
from concourse import bacc, mybir  # noqa: E402
from concourse.bass_utils import run_bass_kernel_spmd  # noqa: E402

# Problem constants (hardcoded per harness contract)
N, C, H, W = 4, 256, 64, 64
NB = 16          # row blocks (4 source rows each)
NWP = 8          # col blocks (8 source cols each)
K = 96           # contraction = 8x12 source window
BF16 = ml_dtypes.bfloat16

_NC_CACHE = {}


def _build_bass():
    nc = bacc.Bacc()
    stat_d = nc.declare_dram_parameter(
        "stat", [NB, K, 2, NWP, 128], mybir.dt.bfloat16, isOutput=False)
    feat_d = nc.declare_dram_parameter(
        "feat", [NB, K, NWP, 128], mybir.dt.bfloat16, isOutput=False)
    out_d = nc.declare_dram_parameter(
        "out", [NB, 128, 2, NWP, 64], mybir.dt.bfloat16, isOutput=True)

    with tile.TileContext(nc) as tc, ExitStack() as ctx:
        sp = ctx.enter_context(tc.tile_pool(name="sp", bufs=1))
        fp = ctx.enter_context(tc.tile_pool(name="fp", bufs=1))
        pp = ctx.enter_context(tc.tile_pool(name="pp", bufs=6, space="PSUM"))
        op = ctx.enter_context(tc.tile_pool(name="op", bufs=3))

        # whole input resident in SBUF: stat 64KB/part, feat 32KB/part
        stat_sb = sp.tile([K, NB, 2, NWP, 128], mybir.dt.bfloat16,
                          name="stat_sb", tag="st")
        feat_sb = fp.tile([K, NB, NWP, 128], mybir.dt.bfloat16,
                          name="feat_sb", tag="ft")
        # all loads queued up front, alternating the two HWDGE rings so
        # each hb slab's two halves arrive in parallel
        for hb in range(NB):
            le = nc.sync if hb % 2 == 0 else nc.scalar
            lo = nc.scalar if hb % 2 == 0 else nc.sync
            le.dma_start(out=stat_sb[:, hb], in_=stat_d[hb])
            lo.dma_start(out=feat_sb[:, hb], in_=feat_d[hb])

        for hb in range(NB):
            ot = op.tile([128, 2, NWP, 64], mybir.dt.bfloat16,
                         name=f"o{hb}", tag="ot")
            for g in range(2):
                ps = pp.tile([128, NWP, 64], mybir.dt.float32,
                             name=f"p{hb}_{g}", tag="ps")
                for wbp in range(NWP):
                    nc.tensor.matmul(
                        out=ps[:, wbp, :],
                        lhsT=stat_sb[:, hb, g, wbp, :],
                        rhs=feat_sb[:, hb, wbp, 64 * g: 64 * g + 64],
                        start=True, stop=True,
                        skip_group_check=True,
                    )
                # drain the full bank with one contiguous 128-partition copy
                if g == 0:
                    nc.vector.tensor_copy(out=ot[:, g], in_=ps)
                else:
                    nc.scalar.copy(out=ot[:, g], in_=ps)
            # stores ride the idle GpSimd SWDGE queue
            nc.gpsimd.dma_start(out=out_d[hb], in_=ot)

    nc.finalize()
    return nc


def _host_shards(features, masks):
    """Build per-core stat/feat arrays (bf16)."""
    in_maps = []
    for core in range(8):
        n, j = core // 2, core % 2
        f = features[n, 128 * j: 128 * (j + 1)]        # [128, 64, 64] f32
        m = masks[n, 50 * j: 50 * j + 50].reshape(2, 25, 128, 128)

        # feature im2col: feat[hb, (r,w''), wbp, c] = Fpad[c, 4hb+r, 8wbp+w'']
        fpad = np.pad(f, ((0, 0), (2, 2), (2, 2)))
        sw = sliding_window_view(fpad, (8, 12), axis=(1, 2))[:, ::4, ::8]
        feat = np.ascontiguousarray(
            sw.transpose(1, 3, 4, 2, 0)).reshape(NB, K, NWP, 128)

        # mask shear: stat[hb, (r,w''), g, wbp, (h',a,w,b)] holds tap
        # (di=r-h', dj=w''-w) of output pixel (2(4hb+h')+a, 2(8wbp+w)+b)
        mm = m.reshape(2, 5, 5, NB, 4, 2, NWP, 8, 2)  # g,di,dj,hb,h,a,wbp,w,b
        stat = np.zeros((NB, 8, 12, 2, NWP, 4, 2, 8, 2), np.float32)
        for di in range(5):
            for dj in range(5):
                for hp in range(4):
                    for w in range(8):
                        stat[:, hp + di, w + dj, :, :, hp, :, w, :] = \
                            mm[:, di, dj, :, hp, :, :, w, :].transpose(
                                1, 0, 3, 2, 4)
        stat = stat.reshape(NB, K, 2, NWP, 128)

        in_maps.append({
            "stat": stat.astype(BF16),
            "feat": feat.astype(BF16),
        })
    return in_maps


def kernel(features, masks, _trace=False):
    features = np.asarray(features, dtype=np.float32)
    masks = np.asarray(masks, dtype=np.float32)

    in_maps = _host_shards(features, masks)

    if "nc" not in _NC_CACHE:
        _NC_CACHE["nc"] = _build_bass()
    nc = _NC_CACHE["nc"]

    res = run_bass_kernel_spmd(nc, in_maps, list(range(8)), trace=_trace)
    kernel._last_result = res

    out = np.empty((N, C, 2 * H, 2 * W), np.float32)
    for core in range(8):
        n, j = core // 2, core % 2
        od = res.results[core]["out"].astype(np.float32)
        od = od.reshape(NB, 4, 2, 8, 2, 2, NWP, 64)  # hb,h',a,w,b,g,wbp,cc
        od = od.transpose(5, 7, 0, 1, 2, 6, 3, 4)    # g,cc,hb,h',a,wbp,w,b
        out[n, 128 * j: 128 * (j + 1)] = od.reshape(128, 128, 128)
    return out


# revision 4
# speedup vs baseline: 1.5636x; 1.2543x over previous
"""CARAFE-naive upsampling (N=4, C=256, H=W=64, k=5, g=4, s=2) on 8 TRN2
NeuronCores.

Strategy
--------
Sharding: core c <- (batch n = c//2, group-pair j = c%2). Each core owns 128
feature channels (2 of the 4 mask groups) of one batch image.

Compute: blocked im2col. The output is tiled into 4x8 source blocks; a
block's 25-tap neighborhood lives in an 8x12 source window (K=96). Per
(tile, group) ONE matmul computes every tap in a single pass:

    psum[(h',a,w,b), c] = sum_{(r,w'')} statT[(r,w''), (h',a,w,b)]
                                      * feat[(r,w''), c]

statT is the host-sheared mask tile (each column holds one output pixel's
25 taps placed at its window offsets; 96/25 = 3.8x inflation over raw
masks); feat is the host-im2col'd feature window (shared by both groups,
N=64 channel columns each). Every psum element is real output: full
[128, 512] PSUM banks drain with one contiguous DVE/ACT copy per
(row-block, group). Everything is SBUF-resident; all 32 load DMAs are
issued up front on the two HWDGE queues so the 512 LDWEIGHTS+MATMUL
stream never waits on HBM; stores ride the otherwise-idle GpSimd SWDGE
queue. ~13.6 MB HBM traffic and 512 PE instructions per core.
"""

import sys

import numpy as np
from numpy.lib.stride_tricks import sliding_window_view

for _p in ("/opt/trn_rl_repo", "/opt/pypackages"):
    if _p not in sys.path:
        sys.path.append(_p)

import ml_dtypes  # noqa: E402
from contextlib import ExitStack  # noqa: E402

import concourse.bass as bass  # noqa: E402
import concourse.tile as tile  # noqa: E402
from concourse import bacc, mybir  # noqa: E402
from concourse.bass_utils import run_bass_kernel_spmd  # noqa: E402

# Problem constants (hardcoded per harness contract)
N, C, H, W = 4, 256, 64, 64
NB = 16          # row blocks (4 source rows each)
NWP = 8          # col blocks (8 source cols each)
K = 96           # contraction = 8x12 source window
BF16 = ml_dtypes.bfloat16
F8E3 = ml_dtypes.float8_e3m4

_NC_CACHE = {}


def _build_bass():
    nc = bacc.Bacc()
    stat_d = nc.declare_dram_parameter(
        "stat", [NB, K, 2, NWP, 128], mybir.dt.float8e3, isOutput=False)
    feat_d = nc.declare_dram_parameter(
        "feat", [NB, K, NWP, 128], mybir.dt.bfloat16, isOutput=False)
    out_d = nc.declare_dram_parameter(
        "out", [NB, 128, 2, NWP, 64], mybir.dt.bfloat16, isOutput=True)

    with tile.TileContext(nc) as tc, ExitStack() as ctx:
        sp = ctx.enter_context(tc.tile_pool(name="sp", bufs=1))
        fp = ctx.enter_context(tc.tile_pool(name="fp", bufs=1))
        pp = ctx.enter_context(tc.tile_pool(name="pp", bufs=6, space="PSUM"))
        op = ctx.enter_context(tc.tile_pool(name="op", bufs=3))

        # whole input resident in SBUF: stat 64KB/part, feat 32KB/part
        stat_sb = sp.tile([K, NB, 2, NWP, 128], mybir.dt.float8e3,
                          name="stat_sb", tag="st")
        feat_sb = fp.tile([K, NB, NWP, 128], mybir.dt.bfloat16,
                          name="feat_sb", tag="ft")
        # all loads queued up front, alternating the two HWDGE rings so
        # each hb slab's two halves arrive in parallel
        for hb in range(NB):
            le = nc.sync if hb % 2 == 0 else nc.scalar
            lo = nc.scalar if hb % 2 == 0 else nc.sync
            le.dma_start(out=stat_sb[:, hb], in_=stat_d[hb])
            lo.dma_start(out=feat_sb[:, hb], in_=feat_d[hb])

        for hb in range(NB):
            ot = op.tile([128, 2, NWP, 64], mybir.dt.bfloat16,
                         name=f"o{hb}", tag="ot")
            for g in range(2):
                ps = pp.tile([128, NWP, 64], mybir.dt.float32,
                             name=f"p{hb}_{g}", tag="ps")
                for wbp in range(NWP):
                    nc.tensor.matmul(
                        out=ps[:, wbp, :],
                        lhsT=stat_sb[:, hb, g, wbp, :],
                        rhs=feat_sb[:, hb, wbp, 64 * g: 64 * g + 64],
                        start=True, stop=True,
                        skip_group_check=True,
                    )
                # drain the full bank with one contiguous 128-partition copy
                if g == 0:
                    nc.vector.tensor_copy(out=ot[:, g], in_=ps)
                else:
                    nc.scalar.copy(out=ot[:, g], in_=ps)
            # stores ride the idle GpSimd SWDGE queue
            nc.gpsimd.dma_start(out=out_d[hb], in_=ot)

    nc.finalize()
    return nc


def _host_shards(features, masks):
    """Build per-core stat/feat arrays (bf16)."""
    in_maps = []
    for core in range(8):
        n, j = core // 2, core % 2
        f = features[n, 128 * j: 128 * (j + 1)]        # [128, 64, 64] f32
        m = masks[n, 50 * j: 50 * j + 50].reshape(2, 25, 128, 128)

        # feature im2col: feat[hb, (r,w''), wbp, c] = Fpad[c, 4hb+r, 8wbp+w'']
        fpad = np.pad(f, ((0, 0), (2, 2), (2, 2)))
        sw = sliding_window_view(fpad, (8, 12), axis=(1, 2))[:, ::4, ::8]
        feat = np.ascontiguousarray(
            sw.transpose(1, 3, 4, 2, 0)).reshape(NB, K, NWP, 128)

        # mask shear: stat[hb, (r,w''), g, wbp, (h',a,w,b)] holds tap
        # (di=r-h', dj=w''-w) of output pixel (2(4hb+h')+a, 2(8wbp+w)+b)
        mm = m.reshape(2, 5, 5, NB, 4, 2, NWP, 8, 2)  # g,di,dj,hb,h,a,wbp,w,b
        stat = np.zeros((NB, 8, 12, 2, NWP, 4, 2, 8, 2), np.float32)
        for di in range(5):
            for dj in range(5):
                for hp in range(4):
                    for w in range(8):
                        stat[:, hp + di, w + dj, :, :, hp, :, w, :] = \
                            mm[:, di, dj, :, hp, :, :, w, :].transpose(
                                1, 0, 3, 2, 4)
        stat = stat.reshape(NB, K, 2, NWP, 128)

        in_maps.append({
            "stat": stat.astype(F8E3),
            "feat": feat.astype(BF16),
        })
    return in_maps


def kernel(features, masks, _trace=False):
    features = np.asarray(features, dtype=np.float32)
    masks = np.asarray(masks, dtype=np.float32)

    in_maps = _host_shards(features, masks)

    if "nc" not in _NC_CACHE:
        _NC_CACHE["nc"] = _build_bass()
    nc = _NC_CACHE["nc"]

    res = run_bass_kernel_spmd(nc, in_maps, list(range(8)), trace=_trace)
    kernel._last_result = res

    out = np.empty((N, C, 2 * H, 2 * W), np.float32)
    for core in range(8):
        n, j = core // 2, core % 2
        od = res.results[core]["out"].astype(np.float32)
        od = od.reshape(NB, 4, 2, 8, 2, 2, NWP, 64)  # hb,h',a,w,b,g,wbp,cc
        od = od.transpose(5, 7, 0, 1, 2, 6, 3, 4)    # g,cc,hb,h',a,wbp,w,b
        out[n, 128 * j: 128 * (j + 1)] = od.reshape(128, 128, 128)
    return out


# revision 6
# speedup vs baseline: 1.7452x; 1.1161x over previous
"""CARAFE-naive upsampling (N=4, C=256, H=W=64, k=5, g=4, s=2) on 8 TRN2
NeuronCores.

Strategy
--------
Sharding: core c <- (batch n = c//2, group-pair j = c%2). Each core owns 128
feature channels (2 of the 4 mask groups) of one batch image.

Compute: blocked im2col. The output is tiled into 4x8 source blocks; a
block's 25-tap neighborhood lives in an 8x12 source window (K=96). Per
(tile, group) ONE matmul computes every tap in a single pass:

    psum[(h',a,w,b), c] = sum_{(r,w'')} statT[(r,w''), (h',a,w,b)]
                                      * feat[(r,w''), c]

statT is the host-sheared mask tile (each column holds one output pixel's
25 taps placed at its window offsets; 96/25 = 3.8x inflation over raw
masks); feat is the host-im2col'd feature window (shared by both groups,
N=64 channel columns each). Every psum element is real output: full
[128, 512] PSUM banks drain with one contiguous DVE/ACT copy per
(row-block, group). Everything is SBUF-resident; all 32 load DMAs are
issued up front on the two HWDGE queues so the 512 LDWEIGHTS+MATMUL
stream never waits on HBM; stores ride the otherwise-idle GpSimd SWDGE
queue. ~13.6 MB HBM traffic and 512 PE instructions per core.
"""

import sys

import numpy as np
from numpy.lib.stride_tricks import sliding_window_view

for _p in ("/opt/trn_rl_repo", "/opt/pypackages"):
    if _p not in sys.path:
        sys.path.append(_p)

import ml_dtypes  # noqa: E402
from contextlib import ExitStack  # noqa: E402

import concourse.bass as bass  # noqa: E402
import concourse.tile as tile  # noqa: E402
from concourse import bacc, mybir  # noqa: E402
from concourse.bass_utils import run_bass_kernel_spmd  # noqa: E402

# Problem constants (hardcoded per harness contract)
N, C, H, W = 4, 256, 64, 64
NB = 16          # row blocks (4 source rows each)
NWP = 8          # col blocks (8 source cols each)
K = 96           # contraction = 8x12 source window
BF16 = ml_dtypes.bfloat16
F8E3 = ml_dtypes.float8_e3m4

_NC_CACHE = {}


def _build_bass():
    nc = bacc.Bacc()
    # hb-pair-major layouts: DMA slab p covers hb = 2p, 2p+1 with the
    # SBUF-matching [K, hb-in-pair, ...] axis order
    stat_d = nc.declare_dram_parameter(
        "stat", [NB // 2, K, 2, 2, NWP, 128], mybir.dt.float8e3,
        isOutput=False)
    feat_d = nc.declare_dram_parameter(
        "feat", [NB // 2, K, 2, NWP, 128], mybir.dt.bfloat16,
        isOutput=False)
    out_d = nc.declare_dram_parameter(
        "out", [NB, 128, 2, NWP, 64], mybir.dt.bfloat16, isOutput=True)

    with tile.TileContext(nc) as tc, ExitStack() as ctx:
        sp = ctx.enter_context(tc.tile_pool(name="sp", bufs=1))
        fp = ctx.enter_context(tc.tile_pool(name="fp", bufs=1))
        pp = ctx.enter_context(tc.tile_pool(name="pp", bufs=6, space="PSUM"))
        op = ctx.enter_context(tc.tile_pool(name="op", bufs=3))

        # whole input resident in SBUF: stat 64KB/part, feat 32KB/part
        stat_sb = sp.tile([K, NB, 2, NWP, 128], mybir.dt.float8e3,
                          name="stat_sb", tag="st")
        feat_sb = fp.tile([K, NB, NWP, 128], mybir.dt.bfloat16,
                          name="feat_sb", tag="ft")
        # All loads ride the sync HWDGE ring, in 2-hb slabs (~390KB each)
        # to keep the issue stream short. Nothing else may share this ring:
        # a copy or store queued behind a load issue would head-of-line
        # block the pipeline (scalar runs copies, gpsimd runs stores).
        for p in range(NB // 2):
            nc.sync.dma_start(out=stat_sb[:, 2 * p: 2 * p + 2],
                              in_=stat_d[p])
            nc.sync.dma_start(out=feat_sb[:, 2 * p: 2 * p + 2],
                              in_=feat_d[p])

        for hb in range(NB):
            ot = op.tile([128, 2, NWP, 64], mybir.dt.bfloat16,
                         name=f"o{hb}", tag="ot")
            for g in range(2):
                ps = pp.tile([128, NWP, 64], mybir.dt.float32,
                             name=f"p{hb}_{g}", tag="ps")
                for wbp in range(NWP):
                    nc.tensor.matmul(
                        out=ps[:, wbp, :],
                        lhsT=stat_sb[:, hb, g, wbp, :],
                        rhs=feat_sb[:, hb, wbp, 64 * g: 64 * g + 64],
                        start=True, stop=True,
                        skip_group_check=True,
                    )
                # drain the full bank with one contiguous 128-partition copy
                if g == 0:
                    nc.vector.tensor_copy(out=ot[:, g], in_=ps)
                else:
                    nc.scalar.copy(out=ot[:, g], in_=ps)
            # stores ride the idle GpSimd SWDGE queue
            nc.gpsimd.dma_start(out=out_d[hb], in_=ot)

    nc.finalize()
    return nc


def _host_shards(features, masks):
    """Build per-core stat/feat arrays (bf16)."""
    in_maps = []
    for core in range(8):
        n, j = core // 2, core % 2
        f = features[n, 128 * j: 128 * (j + 1)]        # [128, 64, 64] f32
        m = masks[n, 50 * j: 50 * j + 50].reshape(2, 25, 128, 128)

        # feature im2col: feat[hb, (r,w''), wbp, c] = Fpad[c, 4hb+r, 8wbp+w'']
        fpad = np.pad(f, ((0, 0), (2, 2), (2, 2)))
        sw = sliding_window_view(fpad, (8, 12), axis=(1, 2))[:, ::4, ::8]
        feat = np.ascontiguousarray(
            sw.transpose(1, 3, 4, 2, 0)).reshape(NB, K, NWP, 128)

        # mask shear: stat[hb, (r,w''), g, wbp, (h',a,w,b)] holds tap
        # (di=r-h', dj=w''-w) of output pixel (2(4hb+h')+a, 2(8wbp+w)+b)
        mm = m.reshape(2, 5, 5, NB, 4, 2, NWP, 8, 2)  # g,di,dj,hb,h,a,wbp,w,b
        stat = np.zeros((NB, 8, 12, 2, NWP, 4, 2, 8, 2), np.float32)
        for di in range(5):
            for dj in range(5):
                for hp in range(4):
                    for w in range(8):
                        stat[:, hp + di, w + dj, :, :, hp, :, w, :] = \
                            mm[:, di, dj, :, hp, :, :, w, :].transpose(
                                1, 0, 3, 2, 4)
        stat = stat.reshape(NB, K, 2, NWP, 128)

        stat = np.ascontiguousarray(
            stat.reshape(NB // 2, 2, K, 2, NWP, 128).transpose(
                0, 2, 1, 3, 4, 5))
        feat = np.ascontiguousarray(
            feat.reshape(NB // 2, 2, K, NWP, 128).transpose(0, 2, 1, 3, 4))
        in_maps.append({
            "stat": stat.astype(F8E3),
            "feat": feat.astype(BF16),
        })
    return in_maps


def kernel(features, masks, _trace=False):
    features = np.asarray(features, dtype=np.float32)
    masks = np.asarray(masks, dtype=np.float32)

    in_maps = _host_shards(features, masks)

    if "nc" not in _NC_CACHE:
        _NC_CACHE["nc"] = _build_bass()
    nc = _NC_CACHE["nc"]

    res = run_bass_kernel_spmd(nc, in_maps, list(range(8)), trace=_trace)
    kernel._last_result = res

    out = np.empty((N, C, 2 * H, 2 * W), np.float32)
    for core in range(8):
        n, j = core // 2, core % 2
        od = res.results[core]["out"].astype(np.float32)
        od = od.reshape(NB, 4, 2, 8, 2, 2, NWP, 64)  # hb,h',a,w,b,g,wbp,cc
        od = od.transpose(5, 7, 0, 1, 2, 6, 3, 4)    # g,cc,hb,h',a,wbp,w,b
        out[n, 128 * j: 128 * (j + 1)] = od.reshape(128, 128, 128)
    return out
